# revision 1
# baseline (speedup 1.0000x reference)
"""Trainium2 Bass kernel for nn_MemoryModule (sparse_attention).

Reference computation (per batch b):
  Low branch:
    mkl (9216, 64) = memory_keys_low[b] as (T*Hl*Wl, Ck)
    qkl (64, 2304) = query_key_low[b]
    A = softmax_over_n(mkl @ qkl * Ck^-0.5)          # (9216, 2304)
    memory = mvl @ A                                  # (128, 2304), mvl = (Cv, T*Hl*Wl)
  High branch:
    g_attn[t] = softmax_over_t(gk[t] @ gv[t].T * Cv^-0.5)   # (Ck, Cv) per t
    qout[t] = g_attn[t] @ qv                          # (64, 576) -> (256, 24, 24)
    qout = bilinear_upsample_2x(qout)                 # (256, 48, 48)
  out = concat([qout, memory.reshape(128, 48, 48)])   # (384, 48, 48)

Sharding: 8 cores = (b in 0..1) x (j in 0..3), where j picks 576 of the 2304
low-branch query columns (= 12 of the 48 output rows). Softmax is over the
key axis, so column blocks are independent -> no collectives.

On-device layout notes:
 - QK matmul:   out[n_tile(128p), m(576)] += mk[c, n_tile].T @ qkl[c, m]; c=64.
   mk is stored two-halves-stacked (128, 36, 128) so DMA uses 128 partitions
   and the second half runs at PE base partition 64.
 - softmax:     exp on ACT (scale fused), denominator via ones-matmul on PE
   accumulating into PSUM (exact fp32 accumulate), broadcast over partitions.
 - AV matmul:   av[cv, m] += mvT[n_tile, cv].T @ exp[n_tile, m], accumulated
   in PSUM over all 72 n_tiles.
 - Upsample is folded into the high branch as a (576hw x 576o) bilinear
   matrix built on the host (exact 0.25/0.75 taps, bf16-representable).
"""

import os
import sys

for _p in ("/opt/trn_rl_repo",):
    if _p not in sys.path and os.path.isdir(_p):
        sys.path.insert(0, _p)

import numpy as np
import ml_dtypes

import concourse.bass as bass
import concourse.tile as tile
from concourse import bacc, mybir
from concourse import bass_utils

BF16 = mybir.dt.bfloat16
F32 = mybir.dt.float32

B, T, Ck, Cv = 2, 4, 64, 128
H, W, Hl, Wl = 24, 24, 48, 48
HW = H * W            # 576
NLOW = T * Hl * Wl    # 9216
MTOT = Hl * Wl        # 2304
MBLK = MTOT // 4      # 576 query columns per core
NT = NLOW // 128      # 72 n-tiles
NHALF = NT // 2       # 36
HWP = 640             # 576 padded to 5*128
NC_CHUNKS = HWP // 128  # 5

SCALE_LOW = float(Ck) ** -0.5   # 0.125
SCALE_HIGH = float(Cv) ** -0.5  # 0.0883883...

_PROGRAM = None
LAST_PERF = {}


def _u1d(n_in, n_out):
    """Half-pixel bilinear interpolation matrix (n_out, n_in), matches
    jax.image.resize(method='bilinear') for upsampling."""
    U = np.zeros((n_out, n_in), dtype=np.float64)
    scale = n_in / n_out
    for i in range(n_out):
        c = (i + 0.5) * scale - 0.5
        f = int(np.floor(c))
        frac = c - f
        lo = min(max(f, 0), n_in - 1)
        hi = min(max(f + 1, 0), n_in - 1)
        U[i, lo] += 1.0 - frac
        U[i, hi] += frac
    return U


def _build_upsample_full():
    """(H*W, Hl*Wl): column (ho*Wl+wo), row (h*W+w)."""
    Uh = _u1d(H, Hl)  # (48, 24)
    Uw = _u1d(W, Wl)  # (48, 24)
    Ufull = np.einsum("oh,pw->hwop", Uh, Uw).reshape(H * W, Hl * Wl)
    return Ufull.astype(np.float32)


def _build_program():
    nc = bacc.Bacc("TRN2", target_bir_lowering=False, debug=False)

    d_qkl2 = nc.dram_tensor("qkl2", (128, MBLK), BF16, kind="ExternalInput")
    d_mk = nc.dram_tensor("mk", (128, NHALF, 128), BF16, kind="ExternalInput")
    d_mvT = nc.dram_tensor("mvT", (128, NT, 128), BF16, kind="ExternalInput")
    d_gkT = nc.dram_tensor("gkT", (128, T, NC_CHUNKS, Ck), BF16, kind="ExternalInput")
    d_gvT = nc.dram_tensor("gvT", (128, T, NC_CHUNKS, Cv), BF16, kind="ExternalInput")
    d_qvT = nc.dram_tensor("qvT", (128, NC_CHUNKS, Cv), BF16, kind="ExternalInput")
    d_uj = nc.dram_tensor("uj", (128, NC_CHUNKS, MBLK), BF16, kind="ExternalInput")
    d_out = nc.dram_tensor("out", (T * Ck + Cv, MBLK), F32, kind="ExternalOutput")

    EXP = mybir.ActivationFunctionType.Exp

    NP = NHALF  # 36 pairs: pair p = n-tiles (p, p+36) at row groups 0 / 64

    with tile.TileContext(nc) as tc:
        from contextlib import ExitStack

        with ExitStack() as ctx:
            cp = ctx.enter_context(tc.tile_pool(name="const", bufs=1))
            wp = ctx.enter_context(tc.tile_pool(name="work", bufs=1))

            # ---- input loads (HWDGE), chunked + ordered so the main loop
            # starts as soon as the first mk/mvT chunks land.
            qkl2_t = cp.tile([128, MBLK], BF16)
            nc.sync.dma_start(qkl2_t[:], d_qkl2.ap()[:, :])
            mk_t = cp.tile([128, NHALF, 128], BF16)
            mvT_t = cp.tile([128, NT, 128], BF16)  # paired: [:, 2p(+1), :]
            gkT_t = cp.tile([128, T, NC_CHUNKS, Ck], BF16)
            gvT_t = cp.tile([128, T, NC_CHUNKS, Cv], BF16)
            qvT_t = cp.tile([128, NC_CHUNKS, Cv], BF16)
            uj_t = cp.tile([128, NC_CHUNKS, MBLK], BF16)

            nc.sync.dma_start(gvT_t[:], d_gvT.ap()[:, :, :, :])
            nc.sync.dma_start(gkT_t[:], d_gkT.ap()[:, :, :, :])
            nc.sync.dma_start(mk_t[:, 0:6, :], d_mk.ap()[:, 0:6, :])
            nc.sync.dma_start(mvT_t[:, 0:12, :], d_mvT.ap()[:, 0:12, :])
            nc.sync.dma_start(qvT_t[:], d_qvT.ap()[:, :, :])
            nc.sync.dma_start(uj_t[:], d_uj.ap()[:, :, :])
            nc.sync.dma_start(mvT_t[:, 12:24, :], d_mvT.ap()[:, 12:24, :])
            nc.sync.dma_start(mvT_t[:, 24:36, :], d_mvT.ap()[:, 24:36, :])
            nc.sync.dma_start(mvT_t[:, 36:48, :], d_mvT.ap()[:, 36:48, :])
            nc.sync.dma_start(mk_t[:, 6:18, :], d_mk.ap()[:, 6:18, :])
            nc.sync.dma_start(mvT_t[:, 48:60, :], d_mvT.ap()[:, 48:60, :])
            nc.sync.dma_start(mk_t[:, 18:36, :], d_mk.ap()[:, 18:36, :])
            nc.sync.dma_start(mvT_t[:, 60:72, :], d_mvT.ap()[:, 60:72, :])

            ones_t = cp.tile([128, 128], BF16)
            nc.gpsimd.memset(ones_t[:], 1.0)
            ones_f = cp.tile([128, 128], F32)
            nc.gpsimd.memset(ones_f[:], 1.0)

            # ================= high branch =================
            with tc.tile_pool(name="hps", bufs=2, space="PSUM") as hps, \
                 tc.tile_pool(name="qvups", bufs=1, space="PSUM") as qvups, \
                 tc.tile_pool(name="qops", bufs=2, space="PSUM") as qops:

                # per-frame channel attention logits: ga[v, k] (transposed)
                ea = []
                for t in range(T):
                    ga = hps.tile([128, Ck], F32, name=f"ga{t}", tag="ga")
                    for c in range(NC_CHUNKS):
                        nc.tensor.matmul(
                            ga[:, :],
                            gvT_t[:, t, c, :],
                            gkT_t[:, t, c, :],
                            start=(c == 0),
                            stop=(c == NC_CHUNKS - 1),
                        )
                    e = wp.tile([128, Ck], F32, name=f"ea{t}", tag=f"ea{t}")
                    nc.scalar.activation(e[:], ga[:], EXP, scale=SCALE_HIGH)
                    ea.append(e)

                s01 = wp.tile([128, Ck], F32)
                nc.vector.tensor_add(s01[:], ea[0][:], ea[1][:])
                s23 = wp.tile([128, Ck], F32)
                nc.vector.tensor_add(s23[:], ea[2][:], ea[3][:])
                ssum = wp.tile([128, Ck], F32)
                nc.vector.tensor_add(ssum[:], s01[:], s23[:])
                rs = wp.tile([128, Ck], F32)
                nc.vector.reciprocal(rs[:], ssum[:])
                wts = []
                for t in range(T):
                    wt = wp.tile([128, Ck], BF16, name=f"wt{t}", tag=f"wt{t}")
                    nc.vector.tensor_mul(wt[:], ea[t][:], rs[:])
                    wts.append(wt)

                # qv_up[v, o] = sum_hw qv[v, hw] * U[hw, o]
                qvup = qvups.tile([128, MBLK], F32)
                for c in range(NC_CHUNKS):
                    st, sp = (c == 0), (c == NC_CHUNKS - 1)
                    nc.tensor.matmul(
                        qvup[:, 0:512], qvT_t[:, c, :], uj_t[:, c, 0:512],
                        start=st, stop=sp,
                    )
                    nc.tensor.matmul(
                        qvup[:, 512:MBLK], qvT_t[:, c, :], uj_t[:, c, 512:MBLK],
                        start=st, stop=sp,
                    )
                qvup_bf = wp.tile([128, MBLK], BF16)
                nc.vector.tensor_copy(qvup_bf[:], qvup[:])

                # qout[t] rows of the output: (64k, 576o)
                for t in range(T):
                    qo = qops.tile([Ck, MBLK], F32, name=f"qo{t}", tag="qo")
                    nc.tensor.matmul(
                        qo[:, 0:512], wts[t][:, :], qvup_bf[:, 0:512],
                        start=True, stop=True,
                    )
                    nc.tensor.matmul(
                        qo[:, 512:MBLK], wts[t][:, :], qvup_bf[:, 512:MBLK],
                        start=True, stop=True,
                    )
                    qo_sb = wp.tile([Ck, MBLK], F32, name=f"qosb{t}", tag="qosb")
                    nc.vector.tensor_copy(qo_sb[:], qo[:])
                    nc.sync.dma_start(d_out.ap()[t * Ck:(t + 1) * Ck, :], qo_sb[:])

            # ================= low branch main loop =================
            # Pair p: QK for n-tiles p (rows 0-63) and p+36 (rows 64-127) run
            # as concurrent row-group matmuls into one bf16 PSUM pair tile
            # (2 banks). exp evacuates both; DVE forms the pair-sum; the
            # denominator is a ones-matmul over pair-sums (36 instead of 72).
            with tc.tile_pool(name="qkps", bufs=2, space="PSUM") as qkps, \
                 tc.tile_pool(name="avps", bufs=1, space="PSUM") as avps, \
                 tc.tile_pool(name="dnps", bufs=1, space="PSUM") as dnps, \
                 tc.tile_pool(name="epool", bufs=6) as epool:

                av = avps.tile([128, MBLK], F32)
                dn = dnps.tile([128, MBLK], F32)

                def emit_qk(q):
                    qk = qkps.tile([128, MBLK], F32, name=f"qk{q}", tag="qk")
                    base = 0 if q < NHALF else 64
                    lhsT = mk_t[base:base + 64, q % NHALF, :]
                    nc.tensor.matmul(
                        qk[:, 0:512], lhsT,
                        qkl2_t[base:base + 64, 0:512],
                        start=True, stop=True,
                    )
                    nc.tensor.matmul(
                        qk[:, 512:MBLK], lhsT,
                        qkl2_t[base:base + 64, 512:MBLK],
                        start=True, stop=True,
                    )
                    return qk

                acc_a = wp.tile([128, MBLK - 512], F32)
                acc_b = wp.tile([128, MBLK - 512], F32)

                nxt = emit_qk(0)
                for q in range(NT):
                    cur = nxt
                    if q + 1 < NT:
                        nxt = emit_qk(q + 1)
                    e = epool.tile([128, MBLK], BF16, name=f"e{q}", tag="e")
                    nc.scalar.activation(e[:], cur[:], EXP, scale=SCALE_LOW)
                    st, sp = (q == 0), (q == NT - 1)
                    nc.tensor.matmul(dn[:, 0:512], ones_t[:, :], e[:, 0:512],
                                     start=st, stop=sp)
                    # 64-col denominator slice: exact fp32 running sum on DVE
                    if q == 0:
                        nc.vector.tensor_copy(acc_a[:], e[:, 512:MBLK])
                    else:
                        s, dst = (acc_a, acc_b) if q % 2 == 1 else (acc_b, acc_a)
                        nc.vector.tensor_add(dst[:], s[:], e[:, 512:MBLK])
                    mvk = mvT_t[:, q, :]
                    nc.tensor.matmul(av[:, 0:512], mvk, e[:, 0:512],
                                     start=st, stop=sp)
                    nc.tensor.matmul(av[:, 512:MBLK], mvk, e[:, 512:MBLK],
                                     start=st, stop=sp)
                # partition-reduce + broadcast of the 64-col slice (fp32 MM)
                nc.tensor.matmul(dn[:, 512:MBLK], ones_f[:, :], acc_b[:],
                                 start=True, stop=True)

                rcp_sb = wp.tile([128, MBLK], F32)
                rcp_scr = wp.tile([128, MBLK], F32)
                mem_sb = wp.tile([128, MBLK], F32)
                r0 = T * Ck
                for lo, hi in ((0, 512), (512, MBLK)):
                    nc.vector.reciprocal_approx_accurate(
                        rcp_sb[:, lo:hi], dn[:, lo:hi], rcp_scr[:, lo:hi])
                    nc.vector.tensor_mul(
                        mem_sb[:, lo:hi], av[:, lo:hi], rcp_sb[:, lo:hi])
                    nc.sync.dma_start(
                        d_out.ap()[r0:r0 + Cv, lo:hi], mem_sb[:, lo:hi])


    nc.compile()
    return nc


def _get_program():
    global _PROGRAM
    if _PROGRAM is None:
        _PROGRAM = _build_program()
    return _PROGRAM


def _prep_core_inputs(memory_keys, memory_values, query_value,
                      memory_keys_low, memory_values_low, query_key_low,
                      Ufull, b, j):
    bf = ml_dtypes.bfloat16

    # ---- low branch
    mk_cn = np.ascontiguousarray(
        memory_keys_low[b].transpose(1, 0, 2, 3).reshape(Ck, NLOW)
    )
    mk2 = np.concatenate([mk_cn[:, : NLOW // 2], mk_cn[:, NLOW // 2:]], axis=0)
    mk2 = np.ascontiguousarray(mk2.reshape(128, NHALF, 128)).astype(bf)

    mv_cn = memory_values_low[b].transpose(1, 0, 2, 3).reshape(Cv, NLOW)
    mvT = np.ascontiguousarray(
        mv_cn.reshape(Cv, NT, 128).transpose(2, 1, 0)
    ).astype(bf)  # (p, k, cv)

    qkl = query_key_low[b].reshape(Ck, MTOT)[:, j * MBLK:(j + 1) * MBLK]
    qkl2 = np.ascontiguousarray(np.concatenate([qkl, qkl], axis=0)).astype(bf)

    # ---- high branch (zero-padded hw -> 640 = 5*128 chunks)
    gk = memory_keys[b].reshape(T, Ck, HW)
    gkp = np.zeros((T, Ck, HWP), np.float32)
    gkp[:, :, :HW] = gk
    gkT = np.ascontiguousarray(
        gkp.reshape(T, Ck, NC_CHUNKS, 128).transpose(3, 0, 2, 1)
    ).astype(bf)  # (p, t, c, k)

    gv = memory_values[b].reshape(T, Cv, HW)
    gvp = np.zeros((T, Cv, HWP), np.float32)
    gvp[:, :, :HW] = gv
    gvT = np.ascontiguousarray(
        gvp.reshape(T, Cv, NC_CHUNKS, 128).transpose(3, 0, 2, 1)
    ).astype(bf)  # (p, t, c, v)

    qv = query_value[b].reshape(Cv, HW)
    qvp = np.zeros((Cv, HWP), np.float32)
    qvp[:, :HW] = qv
    qvT = np.ascontiguousarray(
        qvp.reshape(Cv, NC_CHUNKS, 128).transpose(2, 1, 0)
    ).astype(bf)  # (p, c, v)

    ujf = np.zeros((HWP, MBLK), np.float32)
    ujf[:HW, :] = Ufull[:, j * MBLK:(j + 1) * MBLK]
    uj = np.ascontiguousarray(
        ujf.reshape(NC_CHUNKS, 128, MBLK).transpose(1, 0, 2)
    ).astype(bf)  # (p, c, o)

    return {
        "qkl2": qkl2, "mk": mk2, "mvT": mvT,
        "gkT": gkT, "gvT": gvT, "qvT": qvT, "uj": uj,
    }


def kernel(memory_keys, memory_values, query_value,
           memory_keys_low, memory_values_low, query_key_low):
    memory_keys = np.asarray(memory_keys, dtype=np.float32)
    memory_values = np.asarray(memory_values, dtype=np.float32)
    query_value = np.asarray(query_value, dtype=np.float32)
    memory_keys_low = np.asarray(memory_keys_low, dtype=np.float32)
    memory_values_low = np.asarray(memory_values_low, dtype=np.float32)
    query_key_low = np.asarray(query_key_low, dtype=np.float32)

    Ufull = _build_upsample_full()
    nc = _get_program()

    in_maps = []
    for core in range(8):
        b, j = core // 4, core % 4
        in_maps.append(_prep_core_inputs(
            memory_keys, memory_values, query_value,
            memory_keys_low, memory_values_low, query_key_low, Ufull, b, j))

    trace = os.environ.get("KERNEL_TRACE", "0") == "1"
    kwargs = {}
    if trace and os.environ.get("KERNEL_TRACE_DIR"):
        os.makedirs(os.environ["KERNEL_TRACE_DIR"], exist_ok=True)
        kwargs["tmpdir"] = os.environ["KERNEL_TRACE_DIR"]
    res = bass_utils.run_bass_kernel_spmd(
        nc, in_maps, core_ids=list(range(8)), trace=trace, **kwargs
    )
    LAST_PERF.clear()
    LAST_PERF.update(
        exec_time_ns=res.exec_time_ns,
        mean_exec_time_ns=getattr(res, "mean_exec_time_ns", None),
        max_exec_time_core_id=getattr(res, "max_exec_time_core_id", None),
        per_core_scope_times=getattr(res, "per_core_scope_times", None),
        trace=getattr(res, "instructions_and_trace", None),
    )

    out = np.empty((B, T * Ck + Cv, Hl, Wl), np.float32)
    for core in range(8):
        b, j = core // 4, core % 4
        blk = res.results[core]["out"]  # (384, 576)
        out[b, :, 12 * j:12 * (j + 1), :] = blk.reshape(T * Ck + Cv, 12, Wl)
    return out

